# revision 1
# baseline (speedup 1.0000x reference)
"""Graphormer multi-head attention on 8 TRN2 NeuronCores.

Sharding (2D, data + head parallel): core c -> batch c//4, head-quad c%4
(4 heads per core as 2 pairs).  Per-core DMA: q/k/v only for its batch
(12.6 MB), bias slice 33.5 MB bf16, bf16 partial output 4.2 MB.

 - QKV projections column-parallel per pair (128 of 1024 output dims each).
 - Attention in transposed layout: scoresT = K@Q^T with S on partitions and
   T free.  The two heads of a pair use PE row-tiling (K=64 stationaries at
   partitions 0-63 / 64-127 -> tile_position (0,0)/(64,0)) so their scores
   matmuls can overlap in the PE array.
 - Softmax denominator from a ones column appended to the PV stationary
   (row 64 of the PV accumulator); 1/den via the custom-DVE
   reciprocal_approx_fast, broadcast across partitions on idle GPSIMD.
 - bias (spatial+directional+attn_mask, exp'd and bf16 on the host) applied
   multiplicatively on DVE at 2x rate over [128, 2048] tiles.
 - Out-projection column-parallel over this core's 256 dims; bf16 partials
   summed on the host (the all-reduce) together with bo.

Emission is software-pipelined: a minimal head (first 1024-column chunk of
the q/k/v projections + first half of the V transposes for pair 0), then
the attention st-loops with the remaining projection/transpose work woven
one-or-two items per st-pair, so the ACT engine (the exp wall, ~16.8M
elements/core ~= 142us) starts early and never starves.
"""

import os
from contextlib import ExitStack

import ml_dtypes
import numpy as np

import concourse.bass as bass
import concourse.tile as tile
from concourse import bacc
from concourse import mybir
from concourse.bass_utils import run_bass_kernel_spmd
from concourse.masks import make_identity

B, T, S, E, H, D = 2, 2048, 2048, 1024, 16, 64
NCORES = 8
HPC = 4                    # heads per core
NPAIR = 2                  # head pairs per core
PSL = HPC * D              # per-core projection slice = 256
EC = E // 128              # contraction chunks = 8
ST = S // 128              # s tiles = 16
TCH = 1024                 # t block
NTH = T // TCH             # 2
NB = 512                   # fp32 psum bank free size
BF16 = mybir.dt.bfloat16
F32 = mybir.dt.float32
NPBF16 = ml_dtypes.bfloat16
AF = mybir.ActivationFunctionType

_MODULES = {}
LAST_RUN = None


def build_module():
    key = "main"
    if key in _MODULES:
        return _MODULES[key]

    nc = bacc.Bacc("TRN2", target_bir_lowering=False, debug=False)

    qT_d = nc.dram_tensor("qT", [128, EC, T], BF16, kind="ExternalInput")
    kT_d = nc.dram_tensor("kT", [128, EC, S], BF16, kind="ExternalInput")
    vT_d = nc.dram_tensor("vT", [128, EC, S], BF16, kind="ExternalInput")
    # host layout: [head, th, sp, 128, 2, TCH] (exp'd bias, transposed (s,t))
    bias_d = nc.dram_tensor("biasT", [HPC, NTH, ST // 2, 128, 2, TCH], BF16,
                            kind="ExternalInput")
    wq_d = nc.dram_tensor("wqT", [128, EC, PSL], BF16, kind="ExternalInput")
    wk_d = nc.dram_tensor("wkT", [128, EC, PSL], BF16, kind="ExternalInput")
    wv_d = nc.dram_tensor("wvT", [128, EC, PSL], BF16, kind="ExternalInput")
    wo_d = nc.dram_tensor("woT", [128, NPAIR, E], BF16, kind="ExternalInput")
    bq_d = nc.dram_tensor("bq", [128, NPAIR], F32, kind="ExternalInput")
    bk_d = nc.dram_tensor("bk", [128, NPAIR], F32, kind="ExternalInput")
    bv_d = nc.dram_tensor("bv", [128, NPAIR], F32, kind="ExternalInput")
    y_d = nc.dram_tensor("ypart", [T, E], BF16, kind="ExternalOutput")

    with tile.TileContext(nc) as tc, ExitStack() as ctx:
        consts = ctx.enter_context(tc.tile_pool(name="consts", bufs=1))
        xpool = ctx.enter_context(tc.tile_pool(name="xstage", bufs=1))
        persist = ctx.enter_context(tc.tile_pool(name="persist", bufs=1))
        biasp = ctx.enter_context(tc.tile_pool(name="biasp", bufs=3))
        ptpool = ctx.enter_context(tc.tile_pool(name="ptpool", bufs=4))
        normp = ctx.enter_context(tc.tile_pool(name="normp", bufs=1))
        rdenp = ctx.enter_context(tc.tile_pool(name="rdenp", bufs=1))
        ysp = ctx.enter_context(tc.tile_pool(name="ysp", bufs=2))
        # psum: scores/proj chunks [128,512] f32 = 1 bank x4; acc = 2 banks x2
        psumS = ctx.enter_context(tc.tile_pool(name="psumS", bufs=4, space="PSUM"))
        psumA = ctx.enter_context(tc.tile_pool(name="psumA", bufs=2, space="PSUM"))

        ident = consts.tile([128, 128], BF16, tag="ident", name="ident")
        make_identity(nc, ident[:])
        w_sb = {}
        for nm, wd in (("q", wq_d), ("k", wk_d), ("v", wv_d)):
            w_s = consts.tile([128, EC, PSL], BF16, tag=f"w{nm}", name=f"w{nm}")
            w_sb[nm] = w_s
        wo_s = consts.tile([128, NPAIR, E], BF16, tag="wo", name="wo")
        w_d = {"q": wq_d, "k": wk_d, "v": wv_d}

        def load_w(nm):
            nc.sync.dma_start(w_sb[nm][:], w_d[nm][:])
        b_sb = {}
        for nm, bd in (("q", bq_d), ("k", bk_d), ("v", bv_d)):
            b_s = consts.tile([128, NPAIR], F32, tag=f"b{nm}", name=f"b{nm}")
            nc.sync.dma_start(b_s[:], bd[:])
            b_sb[nm] = b_s

        x_sb, x_d = {}, {"q": qT_d, "k": kT_d, "v": vT_d}
        for nm, L in (("q", T), ("k", S), ("v", S)):
            x_sb[nm] = xpool.tile([128, EC, L], BF16, tag=f"x{nm}", name=f"x{nm}")

        qTs = persist.tile([128, NPAIR, T], BF16, tag="qTs", name="qTs")
        kTs = persist.tile([128, NPAIR, S], BF16, tag="kTs", name="kTs")
        # [s-part, pair, st, head-in-pair, d | ones col]
        vnat = persist.tile([128, NPAIR, ST, 2, 65], BF16, tag="vnat", name="vnat")
        outn = persist.tile([128, NPAIR, T], BF16, tag="outn", name="outn")
        vt_sb = persist.tile([128, NPAIR, S], BF16, tag="vt", name="vt")

        def load_x(nm, c0):
            # 4 triggers so the transfer spreads across DMA queues
            for e0 in range(0, EC, 2):
                nc.sync.dma_start(x_sb[nm][:, e0:e0 + 2, c0:c0 + TCH],
                                  x_d[nm][:, e0:e0 + 2, c0:c0 + TCH])

        def proj_chunk(nm, p, c0, on_act):
            """project x[nm] cols [c0, c0+1024) for pair p."""
            dst = {"q": qTs, "k": kTs, "v": vt_sb}[nm]
            for n0 in range(0, TCH, NB):
                pp = psumS.tile([128, NB], F32, tag="sc", name="sc")
                for ec in range(EC):
                    nc.tensor.matmul(
                        pp[:],
                        w_sb[nm][:, ec, p * 128:(p + 1) * 128],
                        x_sb[nm][:, ec, c0 + n0:c0 + n0 + NB],
                        start=(ec == 0), stop=(ec == EC - 1),
                    )
                if on_act:
                    nc.scalar.activation(dst[:, p, c0 + n0:c0 + n0 + NB], pp[:],
                                         AF.Identity, bias=b_sb[nm][:, p:p + 1])
                else:
                    nc.vector.tensor_scalar_add(
                        dst[:, p, c0 + n0:c0 + n0 + NB], pp[:],
                        b_sb[nm][:, p:p + 1])

        def vtrans2(p, st):
            """transpose vt pair-dims x s-tiles st, st+1 into vnat."""
            for s in (st, st + 1):
                ptp = psumS.tile([128, 128], BF16, tag="sc", name="sc")
                nc.tensor.transpose(ptp[:], vt_sb[:, p, s * 128:(s + 1) * 128],
                                    ident[:])
                nc.vector.tensor_copy(vnat[:, p, s, 0, 0:64], ptp[:, 0:64])
                nc.vector.tensor_copy(vnat[:, p, s, 1, 0:64], ptp[:, 64:128])

        def do_item(it):
            if it[0] == "proj":
                _, nm, p, c0, on_act = it
                proj_chunk(nm, p, c0, on_act)
            elif it[0] == "vt2":
                _, p, st = it
                vtrans2(p, st)
            elif it[0] == "optile":
                _, th_, tt_ = it
                outproj_tile_sc(th_, tt_)
            else:
                _, nm, c0 = it
                load_x(nm, c0)

        def attention_block(p, th, weave, post=None):
            t0 = th * TCH
            pouts = [psumA.tile([65, TCH], F32, tag="acc", name="acc")
                     for _ in range(2)]

            def emit_pv_half(pt2_, sp_, h):
                for j in range(2):
                    st = 2 * sp_ + j
                    for n0 in range(0, TCH, NB):
                        nc.tensor.matmul(
                            pouts[h][:, n0:n0 + NB],
                            vnat[:, p, st, h, :],
                            pt2_[h][:, j * TCH + n0:j * TCH + n0 + NB],
                            start=(st == 0), stop=(st == ST - 1),
                        )

            pending = None
            for sp in range(ST // 2):
                for it in (weave[sp] if sp < len(weave) else []):
                    do_item(it)
                pt2, bt2 = [], []
                for h in range(2):
                    pt2.append(ptpool.tile([128, 2 * TCH], BF16, tag="pt",
                                           name="pt"))
                    bt = biasp.tile([128, 2, TCH], BF16, tag="bias", name="bias")
                    nc.sync.dma_start(bt[:], bias_d[2 * p + h, th, sp, :, :, :])
                    bt2.append(bt)
                    # bt's per-partition bytes are contiguous [2*TCH]
                for j in range(2):
                    st = 2 * sp + j
                    for n0 in range(0, TCH, NB):
                        # the two heads' K=64 matmuls sit in different PE
                        # row-groups and are emitted back-to-back: they
                        # dual-stream (measured ~2x on HW)
                        pscs = []
                        for h in range(2):
                            r0 = 64 * h
                            psc = psumS.tile([128, NB], F32, tag="sc", name="sc")
                            nc.tensor.matmul(
                                psc[:],
                                kTs[r0:r0 + 64, p, st * 128:(st + 1) * 128],
                                qTs[r0:r0 + 64, p, t0 + n0:t0 + n0 + NB],
                                start=True, stop=True,
                                tile_position=(r0, 0),
                            )
                            pscs.append(psc)
                        for h in range(2):
                            nc.scalar.activation(
                                pt2[h][:, j * TCH + n0:j * TCH + n0 + NB],
                                pscs[h][:], AF.Exp)
                # the previous st-pair's PV goes here: its bias-mult overlapped
                # this sp's scores, so the PE never waits on DVE
                if pending is not None:
                    emit_pv_half(pending[0], pending[1], 0)
                    emit_pv_half(pending[0], pending[1], 1)
                if post is not None and sp < len(post):
                    for it in post[sp]:
                        do_item(it)
                for h in range(2):
                    nc.vector.tensor_mul(
                        pt2[h][:].rearrange("p (j u) -> p j u", j=2),
                        pt2[h][:].rearrange("p (j u) -> p j u", j=2),
                        bt2[h][:])
                pending = (pt2, sp)
            emit_pv_half(pending[0], pending[1], 0)
            emit_pv_half(pending[0], pending[1], 1)
            # normalization: r = 1/den via fast NR reciprocal; outn = pout * r
            # (den copied via ACT: partition-shifted PSUM reads and PSUM
            #  sources for the custom DVE op are broken on HW)
            for h in range(2):
                den = rdenp.tile([1, TCH], F32, tag="den", name="den")
                nc.scalar.copy(den[:], pouts[h][64:65, :])
                # (pouts from psumA)
                rden = rdenp.tile([1, TCH], F32, tag="rden", name="rden")
                nc.vector.reciprocal_approx_fast(rden[:], den[:])
                rb = normp.tile([64, TCH], F32, tag="rb", name="rb")
                nc.gpsimd.partition_broadcast(rb[:], rden[:])
                if h == 0:
                    nc.vector.tensor_mul(
                        outn[0:64, p, t0:t0 + TCH], pouts[h][0:64, :], rb[:])
                else:
                    po_s = normp.tile([64, TCH], F32, tag="po", name="po")
                    nc.vector.tensor_copy(po_s[:], pouts[h][0:64, :])
                    nc.vector.tensor_mul(
                        outn[64:128, p, t0:t0 + TCH], po_s[:], rb[:])

        def outproj_tile_sc(th, tt):
            r0 = th * TCH + tt * 128
            ys = ysp.tile([128, E], BF16, tag="ys", name="ys")
            for n0 in range(0, E, NB):
                py = psumS.tile([128, NB], F32, tag="sc", name="sc")
                for p in range(NPAIR):
                    nc.tensor.matmul(
                        py[:],
                        outn[:, p, r0:r0 + 128],
                        wo_s[:, p, n0:n0 + NB],
                        start=(p == 0), stop=(p == NPAIR - 1),
                    )
                nc.vector.tensor_copy(ys[:, n0:n0 + NB], py[:])
            nc.sync.dma_start(y_d[r0:r0 + 128, :], ys[:])

        def outproj_block(th):
            t0 = th * TCH
            for tt in range(TCH // 128):
                r0 = t0 + tt * 128
                py = psumA.tile([128, E], F32, tag="acc", name="acc")
                for n0 in range(0, E, NB):
                    for p in range(NPAIR):
                        nc.tensor.matmul(
                            py[:, n0:n0 + NB],
                            outn[:, p, r0:r0 + 128],
                            wo_s[:, p, n0:n0 + NB],
                            start=(p == 0), stop=(p == NPAIR - 1),
                        )
                ys = ysp.tile([128, E], BF16, tag="ys", name="ys")
                nc.vector.tensor_copy(ys[:], py[:])
                nc.sync.dma_start(y_d[r0:r0 + 128, :], ys[:])

        # ---------------- head: minimal pair-0 prep ----------------
        load_x("q", 0)
        load_w("q")
        load_x("k", 0)
        load_w("k")
        load_x("v", 0)
        load_w("v")
        nc.sync.dma_start(wo_s[:], wo_d[:])
        proj_chunk("q", 0, 0, on_act=True)
        proj_chunk("k", 0, 0, on_act=True)
        proj_chunk("v", 0, 0, on_act=True)
        nc.vector.memset(vnat[:, :, :, :, 64:65], 1.0)
        for st in (0, 2, 4, 6):
            vtrans2(0, st)

        # ---------------- woven attention schedule ----------------
        w00 = [
            [("load", "k", TCH)],
            [("load", "v", TCH), ("proj", "k", 0, TCH, False)],
            [("proj", "v", 0, TCH, False)],
            [("vt2", 0, 8)],
            [("vt2", 0, 10), ("load", "q", TCH)],
            [("vt2", 0, 12)],
            [("vt2", 0, 14)],
            [("proj", "q", 0, TCH, False)],
        ]
        w01 = [
            [("proj", "q", 1, 0, False)],
            [("proj", "q", 1, TCH, False)],
            [("proj", "k", 1, 0, False)],
            [("proj", "k", 1, TCH, False)],
            [("proj", "v", 1, 0, False)],
            [("proj", "v", 1, TCH, False)],
            [("vt2", 1, 0), ("vt2", 1, 2)],
            [("vt2", 1, 4), ("vt2", 1, 6)],
        ]
        w10 = [
            [("vt2", 1, 8)],
            [("vt2", 1, 10)],
            [("vt2", 1, 12)],
            [("vt2", 1, 14)],
        ]
        post11 = [[]] + [[("optile", 0, tt)] for tt in range(8)][:7]
        attention_block(0, 0, w00)
        attention_block(0, 1, w01)
        attention_block(1, 0, w10)
        outproj_tile_sc(0, 7)
        attention_block(1, 1, [], post=post11)
        outproj_block(1)

    nc.compile()
    _MODULES[key] = nc
    return nc


def make_in_maps(query, key, value, spatial_bias, directional_bias,
                 key_padding_mask, attn_mask, Wq, bq, Wk, bk, Wv, bv, Wo, bo):
    scale = D ** -0.5
    def prep_x(x):
        # [T, E] -> transposed [E, T] -> [128, EC, T] (partition-major)
        xt = np.ascontiguousarray(x.T, dtype=NPBF16)
        return np.ascontiguousarray(
            xt.reshape(EC, 128, T).transpose(1, 0, 2))
    qT = [prep_x(query[b]) for b in range(B)]
    kT = [prep_x(key[b]) for b in range(B)]
    vT = [prep_x(value[b]) for b in range(B)]
    pad_any = bool(np.any(key_padding_mask))
    in_maps = []
    for c in range(NCORES):
        b = c // 4
        h0 = (c % 4) * HPC
        sl = slice(h0 * D, (h0 + HPC) * D)
        bias = spatial_bias[b, h0:h0 + HPC].astype(np.float32) \
            + directional_bias[b, h0:h0 + HPC]
        bias += attn_mask[None]
        if pad_any:
            bias = np.where(key_padding_mask[b, None, None, :], -1e30, bias)
        np.exp(bias, out=bias)  # kernel applies bias multiplicatively
        # [h, T, S] -> [h, S, T] -> [h, th, sp, 128, 2, TCH]
        biasT = np.ascontiguousarray(bias.transpose(0, 2, 1), dtype=NPBF16)
        biasT = np.ascontiguousarray(
            biasT.reshape(HPC, ST // 2, 2, 128, NTH, TCH)
            .transpose(0, 4, 1, 3, 2, 5))
        in_maps.append({
            "qT": qT[b], "kT": kT[b], "vT": vT[b], "biasT": biasT,
            "wqT": np.ascontiguousarray(np.ascontiguousarray((Wq[sl, :].T * scale), dtype=NPBF16).reshape(EC, 128, PSL).transpose(1, 0, 2)),
            "wkT": np.ascontiguousarray(np.ascontiguousarray(Wk[sl, :].T, dtype=NPBF16).reshape(EC, 128, PSL).transpose(1, 0, 2)),
            "wvT": np.ascontiguousarray(np.ascontiguousarray(Wv[sl, :].T, dtype=NPBF16).reshape(EC, 128, PSL).transpose(1, 0, 2)),
            "woT": np.ascontiguousarray(np.ascontiguousarray(Wo[:, sl].T, dtype=NPBF16).reshape(NPAIR, 128, E).transpose(1, 0, 2)),
            "bq": (bq[sl] * scale).reshape(NPAIR, 128).T.astype(np.float32).copy(),
            "bk": bk[sl].reshape(NPAIR, 128).T.astype(np.float32).copy(),
            "bv": bv[sl].reshape(NPAIR, 128).T.astype(np.float32).copy(),
        })
    return in_maps


def _install_ntff_shim():
    """bass_utils' trace path imports antenv.axon_hooks, which this image
    lacks; synthesize it around trn_boot's ctypes NTFF hook."""
    import sys
    import types
    if "antenv.axon_hooks" in sys.modules:
        return
    try:
        import antenv
        from trn_agent_boot.trn_boot import _ntff_profile_via_ctypes
        hook = _ntff_profile_via_ctypes("/opt/axon/libaxon_pjrt.so")
        mod = types.ModuleType("antenv.axon_hooks")
        mod._hook = hook
        mod.get_axon_ntff_profile_hook = lambda: mod._hook
        mod.set_axon_ntff_profile_hook = lambda h: setattr(mod, "_hook", h)
        sys.modules["antenv.axon_hooks"] = mod
        antenv.axon_hooks = mod
    except Exception as exc:  # pragma: no cover
        print("ntff shim unavailable:", exc)


def kernel(**inputs):
    global LAST_RUN
    if os.environ.get("BASS_TRACE"):
        _install_ntff_shim()
    nc = build_module()
    in_maps = make_in_maps(**inputs)
    res = run_bass_kernel_spmd(
        nc, in_maps, core_ids=list(range(NCORES)),
        trace=bool(os.environ.get("BASS_TRACE")),
    )
    LAST_RUN = res
    bo = inputs["bo"]
    y = np.zeros((B, T, E), dtype=np.float64)
    for c in range(NCORES):
        y[c // 4] += res.results[c]["ypart"].astype(np.float64)
    y += bo
    return y.astype(np.float32)



# revision 10
# speedup vs baseline: 1.0188x; 1.0188x over previous
"""Graphormer multi-head attention on 8 TRN2 NeuronCores.

Sharding (2D, data + head parallel): core c -> batch c//4, head-quad c%4
(4 heads per core as 2 pairs).  Per-core DMA: q/k/v only for its batch
(12.6 MB), bias slice 33.5 MB bf16, bf16 partial output 4.2 MB.

v1 design (vs the 292 us baseline):
 - t-blocks of 512 (8 blocks: (p0,th0),(p0,th1),(p1,th0),(p1,th1),
   (p0,th2),(p1,th2),(p0,th3),(p1,th3)).
 - scores per (head, st) land in a [128, 2, 512] fp32 PSUM tile (2 banks,
   ping-ponged pool of 2 => 4 banks); ONE exp per st covers both heads
   (FD=1024, amortizing the ~352-cycle ACT per-instruction overhead:
   256x512-elem exps @720ns -> 128x1024-elem @1147ns, ~45 us saved).
 - PV accumulates into ONE [65, 2, 512] fp32 tile (2 banks, ones row at
   partition 64 for the softmax denominator); PV emission lags scores by
   LAG st-tiles so the previous block's normalization can drain pouts.
 - remaining 2 PSUM banks are the weave pool (QKV projection chunks and
   out-projection tiles interleave into the st-loops).
 - V transposes moved off the PE: SBUF->SBUF xbar dma_start_transpose to
   a [128, 4, 128] scratch (row r -> partition r%128, tile r//128), then
   one strided 2x DVE copy into vnat.
 - DMA split across 3 queues: bias stream -> sync (SP), x/weights (+
   transposes) -> scalar, output stores -> gpsimd SWDGE.  Baseline pushed
   all 52 MB through the single sync queue at ~180 GB/s average.
 - host x layout [128, chunk, EC, 512] so each x DMA is 1 MB with 8 KB
   per-partition contiguous lines.
"""

import os
from contextlib import ExitStack

import ml_dtypes
import numpy as np

import concourse.bass as bass
import concourse.tile as tile
from concourse import bacc
from concourse import mybir
from concourse.bass_utils import run_bass_kernel_spmd

B, T, S, E, H, D = 2, 2048, 2048, 1024, 16, 64
NCORES = 8
HPC = 4                    # heads per core
NPAIR = 2                  # head pairs per core
PSL = HPC * D              # per-core projection slice = 256
EC = E // 128              # contraction chunks = 8
ST = S // 128              # s tiles = 16
TCH = 512                  # t block
NTH = T // TCH             # 4
NCH = T // TCH             # x column chunks = 4
NB = 512                   # fp32 psum bank free size
LAG = 4                    # PV lags scores by this many st tiles
BF16 = mybir.dt.bfloat16
F32 = mybir.dt.float32
NPBF16 = ml_dtypes.bfloat16
AF = mybir.ActivationFunctionType

_MODULES = {}
LAST_RUN = None


def build_module():
    key = "main"
    if key in _MODULES:
        return _MODULES[key]

    nc = bacc.Bacc("TRN2", target_bir_lowering=False, debug=False)

    qT_d = nc.dram_tensor("qT", [128, NCH, EC, TCH], BF16, kind="ExternalInput")
    kT_d = nc.dram_tensor("kT", [128, NCH, EC, TCH], BF16, kind="ExternalInput")
    vT_d = nc.dram_tensor("vT", [128, NCH, EC, TCH], BF16, kind="ExternalInput")
    # host layout: [th, pair, sp2, 128, j(st in pair), h(in pair), t]
    bias_d = nc.dram_tensor("biasT", [NTH, NPAIR, ST // 2, 128, 2, 2, TCH],
                            BF16, kind="ExternalInput")
    wq_d = nc.dram_tensor("wqT", [128, EC, PSL], BF16, kind="ExternalInput")
    wk_d = nc.dram_tensor("wkT", [128, EC, PSL], BF16, kind="ExternalInput")
    wv_d = nc.dram_tensor("wvT", [128, EC, PSL], BF16, kind="ExternalInput")
    wo_d = nc.dram_tensor("woT", [128, NPAIR, E], BF16, kind="ExternalInput")
    bq_d = nc.dram_tensor("bq", [128, NPAIR], F32, kind="ExternalInput")
    bk_d = nc.dram_tensor("bk", [128, NPAIR], F32, kind="ExternalInput")
    bv_d = nc.dram_tensor("bv", [128, NPAIR], F32, kind="ExternalInput")
    y_d = nc.dram_tensor("ypart", [T, E], BF16, kind="ExternalOutput")

    with tile.TileContext(nc) as tc, ExitStack() as ctx:
        consts = ctx.enter_context(tc.tile_pool(name="consts", bufs=1))
        xpool = ctx.enter_context(tc.tile_pool(name="xstage", bufs=1))
        persist = ctx.enter_context(tc.tile_pool(name="persist", bufs=1))
        biasp = ctx.enter_context(tc.tile_pool(name="biasp", bufs=3))
        ptp = ctx.enter_context(tc.tile_pool(name="ptp", bufs=7))
        vscrp = ctx.enter_context(tc.tile_pool(name="vscrp", bufs=2))
        normp = ctx.enter_context(tc.tile_pool(name="normp", bufs=2))
        rdenp = ctx.enter_context(tc.tile_pool(name="rdenp", bufs=1))
        ysp = ctx.enter_context(tc.tile_pool(name="ysp", bufs=2))
        # PSUM: scores 2x[128,2,512]f32 (4 banks) + pv 1x[65,2,512]f32
        # (2 banks) + weave 2x[128,512]f32 (2 banks) = 8 banks exactly
        scp = ctx.enter_context(tc.tile_pool(name="scp", bufs=2, space="PSUM"))
        povp = ctx.enter_context(tc.tile_pool(name="povp", bufs=1, space="PSUM"))
        wvp = ctx.enter_context(tc.tile_pool(name="wvp", bufs=2, space="PSUM"))

        w_sb = {}
        for nm in ("q", "k", "v"):
            w_sb[nm] = consts.tile([128, EC, PSL], BF16, tag=f"w{nm}",
                                   name=f"w{nm}")
        wo_s = consts.tile([128, NPAIR, E], BF16, tag="wo", name="wo")
        b_sb = {}
        for nm in ("q", "k", "v"):
            b_sb[nm] = consts.tile([128, NPAIR], F32, tag=f"b{nm}",
                                   name=f"b{nm}")
        w_d = {"q": wq_d, "k": wk_d, "v": wv_d}
        b_d = {"q": bq_d, "k": bk_d, "v": bv_d}

        x_sb, x_d = {}, {"q": qT_d, "k": kT_d, "v": vT_d}
        for nm in ("q", "k", "v"):
            x_sb[nm] = xpool.tile([128, NCH, EC, TCH], BF16, tag=f"x{nm}",
                                  name=f"x{nm}")

        qTs = persist.tile([128, NPAIR, T], BF16, tag="qTs", name="qTs")
        kTs = persist.tile([128, NPAIR, S], BF16, tag="kTs", name="kTs")
        # [s-part, pair, st, head-in-pair, d | ones col at 64]
        vnat = persist.tile([128, NPAIR, ST, 2, 65], BF16, tag="vnat",
                            name="vnat")
        outn = persist.tile([128, NPAIR, T], BF16, tag="outn", name="outn")
        vt_sb = persist.tile([128, NPAIR, S], BF16, tag="vt", name="vt")

        def load_w(nm):
            nc.scalar.dma_start(w_sb[nm][:], w_d[nm][:])
            nc.scalar.dma_start(b_sb[nm][:], b_d[nm][:])

        def load_x(nm, c):
            nc.scalar.dma_start(x_sb[nm][:, c], x_d[nm][:, c])

        def proj_chunk(nm, p, c, on_act=False):
            """project x[nm] cols [c*512, (c+1)*512) for pair p."""
            dst = {"q": qTs, "k": kTs, "v": vt_sb}[nm]
            pp = wvp.tile([128, NB], F32, tag="wv", name="wv")
            for ec in range(EC):
                nc.tensor.matmul(
                    pp[:],
                    w_sb[nm][:, ec, p * 128:(p + 1) * 128],
                    x_sb[nm][:, c, ec, :],
                    start=(ec == 0), stop=(ec == EC - 1),
                )
            if on_act:
                nc.scalar.activation(dst[:, p, c * TCH:(c + 1) * TCH], pp[:],
                                     AF.Identity, bias=b_sb[nm][:, p:p + 1])
            else:
                nc.vector.tensor_scalar_add(
                    dst[:, p, c * TCH:(c + 1) * TCH], pp[:],
                    b_sb[nm][:, p:p + 1])

        def vtrans(p, c):
            """xbar-transpose vt chunk c of pair p into vnat st 4c..4c+3."""
            vs = vscrp.tile([128, 4, 128], BF16, tag="vs", name="vs")
            nc.scalar.dma_start_transpose(
                vs[:], vt_sb[:, p, c * TCH:(c + 1) * TCH])
            nc.vector.tensor_copy(
                vnat[:, p, 4 * c:4 * c + 4, :, 0:64],
                vs[:].rearrange("s j (h d) -> s j h d", h=2))

        def outproj_tile(th, tt):
            r0 = th * TCH + tt * 128
            ys = ysp.tile([128, E], BF16, tag="ys", name="ys")
            for n0 in range(0, E, NB):
                py = wvp.tile([128, NB], F32, tag="wv", name="wv")
                for p in range(NPAIR):
                    nc.tensor.matmul(
                        py[:],
                        outn[:, p, r0:r0 + 128],
                        wo_s[:, p, n0:n0 + NB],
                        start=(p == 0), stop=(p == NPAIR - 1),
                    )
                nc.vector.tensor_copy(ys[:, n0:n0 + NB], py[:])
            nc.gpsimd.dma_start(y_d[r0:r0 + 128, :], ys[:])

        def do_item(it):
            if it[0] == "proj":
                _, nm, p, c = it
                proj_chunk(nm, p, c)
            elif it[0] == "vt":
                _, p, c = it
                vtrans(p, c)
            elif it[0] == "op":
                _, th_, tt_ = it
                outproj_tile(th_, tt_)

        # pending[0] = (p, th, pouts, ptl) of the previous block, whose last
        # LAG PVs + normalization are woven into the NEXT block's first
        # slots (avoids the tail PV burst delaying the next block's scores,
        # and pouts is freed within ~1.2us by a DVE evacuation to SBUF).
        pending = []

        def emit_pv(p, st, ptl, pouts):
            for h in range(2):
                nc.tensor.matmul(
                    pouts[:, h, :],
                    vnat[:, p, st, h, :],
                    ptl[st][:, h, :],
                    start=(st == 0), stop=(st == ST - 1),
                )

        def finish_pending(slot):
            """Emit deferred tail work of the previous block at `slot` of
            the current block (2 PV st-tiles per slot; evac+norm after)."""
            if not pending:
                return
            p, th, pouts, ptl = pending[0]
            t0 = th * TCH
            if slot < 2:
                for st in range(ST - LAG + 2 * slot, ST - LAG + 2 * slot + 2):
                    emit_pv(p, st, ptl, pouts)
                return
            pending.pop(0)
            # evacuate pouts (frees the PSUM banks for this block's PV)
            pc = normp.tile([65, 2, TCH], F32, tag="pc", name="pc")
            nc.vector.tensor_copy(pc[:], pouts[:])
            # den to partition 0 first: custom-DVE ops cannot take
            # partition-shifted sources
            den = rdenp.tile([1, 2, TCH], F32, tag="den", name="den")
            nc.vector.tensor_copy(den[:], pc[64:65, :, :])
            rden = rdenp.tile([1, 2, TCH], F32, tag="rden", name="rden")
            nc.vector.reciprocal_approx_fast(rden[:], den[:])
            for h in range(2):
                rb = normp.tile([64, TCH], F32, tag="rb", name="rb")
                nc.gpsimd.partition_broadcast(rb[:], rden[:, h, :])
                nc.vector.tensor_mul(
                    outn[64 * h:64 * h + 64, p, t0:t0 + TCH],
                    pc[0:64, h, :], rb[:])

        def attention_block(p, th, weave):
            t0 = th * TCH
            pouts = None  # allocated lazily at st == LAG, after the previous
            # generation's deferred evacuation (povp bufs=1 aliases memory)
            ptl = []
            btl = {}
            for st in range(ST):
                if st % 2 == 0:
                    bt = biasp.tile([128, 2, 2, TCH], BF16, tag="bias",
                                    name="bias")
                    nc.sync.dma_start(bt[:], bias_d[th, p, st // 2])
                    btl[st // 2] = bt
                sc = scp.tile([128, 2, TCH], F32, tag="sc", name="sc")
                for h in range(2):
                    nc.tensor.matmul(
                        sc[:, h, :],
                        kTs[64 * h:64 * h + 64, p, st * 128:(st + 1) * 128],
                        qTs[64 * h:64 * h + 64, p, t0:t0 + TCH],
                        start=True, stop=True,
                        tile_position=(64 * h, 0),
                    )
                pt = ptp.tile([128, 2, TCH], BF16, tag="pt", name="pt")
                nc.scalar.activation(pt[:], sc[:], AF.Exp)
                nc.vector.tensor_mul(pt[:], pt[:], btl[st // 2][:, st % 2])
                ptl.append(pt)
                if st <= 2:
                    finish_pending(st)
                for it in weave.get(st, []):
                    do_item(it)
                if st >= LAG:
                    if pouts is None:
                        pouts = povp.tile([65, 2, TCH], F32, tag="acc",
                                          name="acc")
                    emit_pv(p, st - LAG, ptl, pouts)
            pending.append((p, th, pouts, ptl))

        def flush_all():
            while pending:
                for slot in range(3):
                    finish_pending(slot)

        # ---------------- head: DMA issue order + minimal prep ------------
        nc.vector.memset(vnat[:, :, :, :, 64:65], 1.0)
        # prime the ACT exp table load (~2.7us) during the initial DMA wait
        dumm = consts.tile([1, 16], F32, tag="dumm", name="dumm")
        nc.vector.memset(dumm[:], 0.0)
        nc.scalar.activation(dumm[:], dumm[:], AF.Exp)
        load_w("q")
        load_x("q", 0)
        load_w("k")
        load_x("k", 0)
        load_w("v")
        load_x("v", 0)
        load_x("q", 1)
        load_x("k", 1)
        load_x("v", 1)
        load_x("k", 2)
        load_x("v", 2)
        load_x("k", 3)
        load_x("v", 3)
        load_x("q", 2)
        load_x("q", 3)
        nc.scalar.dma_start(wo_s[:], wo_d[:])

        proj_chunk("q", 0, 0, on_act=True)
        proj_chunk("k", 0, 0, on_act=True)

        # ---------------- woven attention schedule ------------------------
        w1 = {0: [("proj", "v", 0, 0)], 2: [("vt", 0, 0)],
              3: [("proj", "k", 0, 1)], 5: [("proj", "v", 0, 1)],
              6: [("vt", 0, 1)], 7: [("proj", "k", 0, 2)],
              9: [("proj", "v", 0, 2)], 10: [("vt", 0, 2)],
              11: [("proj", "k", 0, 3)], 13: [("proj", "v", 0, 3)],
              14: [("vt", 0, 3)], 15: [("proj", "q", 0, 1)]}
        w2 = {0: [("proj", "k", 1, 0)], 2: [("proj", "v", 1, 0)],
              4: [("vt", 1, 0)], 6: [("proj", "k", 1, 1)],
              8: [("proj", "v", 1, 1)], 10: [("vt", 1, 1)],
              12: [("proj", "k", 1, 2)], 14: [("proj", "q", 1, 0)]}
        w3 = {0: [("proj", "v", 1, 2)], 2: [("vt", 1, 2)],
              4: [("proj", "k", 1, 3)], 6: [("proj", "v", 1, 3)],
              8: [("vt", 1, 3)], 12: [("proj", "q", 1, 1)]}
        w4 = {2: [("op", 0, 0)], 5: [("op", 0, 1)], 8: [("op", 0, 2)],
              11: [("op", 0, 3)], 14: [("proj", "q", 0, 2)]}
        w5 = {2: [("op", 1, 0)], 5: [("op", 1, 1)], 8: [("op", 1, 2)],
              11: [("op", 1, 3)], 14: [("proj", "q", 1, 2)]}
        w6 = {4: [("proj", "q", 0, 3)], 10: [("proj", "q", 1, 3)]}
        w7 = {2: [("op", 2, 0)], 5: [("op", 2, 1)], 8: [("op", 2, 2)],
              11: [("op", 2, 3)]}
        w8 = {}

        attention_block(0, 0, w1)
        attention_block(0, 1, w2)
        attention_block(1, 0, w3)
        attention_block(1, 1, w4)
        attention_block(0, 2, w5)
        attention_block(1, 2, w6)
        attention_block(0, 3, w7)
        attention_block(1, 3, w8)
        flush_all()
        for tt in range(4):
            outproj_tile(3, tt)

    nc.compile()
    _MODULES[key] = nc
    return nc


def make_in_maps(query, key, value, spatial_bias, directional_bias,
                 key_padding_mask, attn_mask, Wq, bq, Wk, bk, Wv, bv, Wo, bo):
    scale = D ** -0.5

    def prep_x(x):
        # [T, E] -> [E, T] -> [128, NCH, EC, TCH] (8KB contiguous/partition)
        xt = np.ascontiguousarray(x.T, dtype=NPBF16)          # [E, T]
        return np.ascontiguousarray(
            xt.reshape(EC, 128, NCH, TCH).transpose(1, 2, 0, 3))

    qT = [prep_x(query[b]) for b in range(B)]
    kT = [prep_x(key[b]) for b in range(B)]
    vT = [prep_x(value[b]) for b in range(B)]
    pad_any = bool(np.any(key_padding_mask))
    in_maps = []
    for c in range(NCORES):
        b = c // 4
        h0 = (c % 4) * HPC
        sl = slice(h0 * D, (h0 + HPC) * D)
        bias = spatial_bias[b, h0:h0 + HPC].astype(np.float32) \
            + directional_bias[b, h0:h0 + HPC]
        bias += attn_mask[None]
        if pad_any:
            bias = np.where(key_padding_mask[b, None, None, :], -1e30, bias)
        np.exp(bias, out=bias)  # kernel applies bias multiplicatively
        # [h, T, S] -> [h, S, T] -> [NTH, NPAIR, sp2, 128, j, h, TCH]
        biasT = np.ascontiguousarray(bias.transpose(0, 2, 1), dtype=NPBF16)
        big = np.empty([NTH, NPAIR, ST // 2, 128, 2, 2, TCH], dtype=NPBF16)
        for p in range(NPAIR):
            for hip in range(2):
                # [S, T] -> (sp2, j, s128, th, tt) -> (th, sp2, s128, j, tt)
                arr = biasT[2 * p + hip].reshape(ST // 2, 2, 128, NTH, TCH)
                big[:, p, :, :, :, hip, :] = arr.transpose(3, 0, 2, 1, 4)
        in_maps.append({
            "qT": qT[b], "kT": kT[b], "vT": vT[b], "biasT": big,
            "wqT": np.ascontiguousarray(np.ascontiguousarray((Wq[sl, :].T * scale), dtype=NPBF16).reshape(EC, 128, PSL).transpose(1, 0, 2)),
            "wkT": np.ascontiguousarray(np.ascontiguousarray(Wk[sl, :].T, dtype=NPBF16).reshape(EC, 128, PSL).transpose(1, 0, 2)),
            "wvT": np.ascontiguousarray(np.ascontiguousarray(Wv[sl, :].T, dtype=NPBF16).reshape(EC, 128, PSL).transpose(1, 0, 2)),
            "woT": np.ascontiguousarray(np.ascontiguousarray(Wo[:, sl].T, dtype=NPBF16).reshape(NPAIR, 128, E).transpose(1, 0, 2)),
            "bq": (bq[sl] * scale).reshape(NPAIR, 128).T.astype(np.float32).copy(),
            "bk": bk[sl].reshape(NPAIR, 128).T.astype(np.float32).copy(),
            "bv": bv[sl].reshape(NPAIR, 128).T.astype(np.float32).copy(),
        })
    return in_maps


def _install_ntff_shim():
    """bass_utils' trace path imports antenv.axon_hooks, which this image
    lacks; synthesize it around trn_boot's ctypes NTFF hook."""
    import sys
    import types
    if "antenv.axon_hooks" in sys.modules:
        return
    try:
        import antenv
        from trn_agent_boot.trn_boot import _ntff_profile_via_ctypes
        hook = _ntff_profile_via_ctypes("/opt/axon/libaxon_pjrt.so")
        mod = types.ModuleType("antenv.axon_hooks")
        mod._hook = hook
        mod.get_axon_ntff_profile_hook = lambda: mod._hook
        mod.set_axon_ntff_profile_hook = lambda h: setattr(mod, "_hook", h)
        sys.modules["antenv.axon_hooks"] = mod
        antenv.axon_hooks = mod
    except Exception as exc:  # pragma: no cover
        print("ntff shim unavailable:", exc)


def kernel(**inputs):
    global LAST_RUN
    if os.environ.get("BASS_TRACE"):
        _install_ntff_shim()
    nc = build_module()
    in_maps = make_in_maps(**inputs)
    res = run_bass_kernel_spmd(
        nc, in_maps, core_ids=list(range(NCORES)),
        trace=bool(os.environ.get("BASS_TRACE")),
    )
    LAST_RUN = res
    bo = inputs["bo"]
    y = np.zeros((B, T, E), dtype=np.float64)
    for c in range(NCORES):
        y[c // 4] += res.results[c]["ypart"].astype(np.float64)
    y += bo
    return y.astype(np.float32)
